# revision 40
# baseline (speedup 1.0000x reference)
"""Trainium2 Bass kernel for a dense transformer block (B=2, T=2048, D=1024,
H=16, hd=64, MLP=4x), distributed across 8 NeuronCores.

Sharding: data-parallel over batch (cores 0-3 = batch 0, cores 4-7 = batch 1)
x tensor-parallel over heads (4 heads/core) for QKV+attention. Re-sharding to
token-parallel for wout/LN2/MLP happens with FOUR per-span bf16
ReduceScatters over the wout partial products, each issued as soon as its
query span finishes attention so the collectives overlap the remaining
attention compute. Core r of a group owns token rows {512*sp + 128*r} for
sp in 0..3 (interleaved), which is exactly what the per-span RS scatters.

Matmul operands are bf16. Weights (wqkv/wout/w1/w2) are pre-cast to bf16 on
the host, halving their HBM traffic; w2 is fully resident in SBUF (prefetched
at kernel start), w1 is streamed during the first MLP matmul.

Softmax: scores are computed as S^T [k, q] blocks, two 512-wide blocks per
PSUM pair so each Exp activation covers 1024 columns (amortizing the ~352
cycle activation overhead). Fully-masked key blocks are skipped at trace
time; partially-masked (diagonal) blocks add a small constant mask tile
(deduplicated on host -- 4 unique tiles for causal) so no [T,T] mask is ever
DMA'd. The PV matmul carries an appended ones-column producing softmax
denominators in the same PSUM accumulation; normalization multiplies by a
reciprocal row broadcast across partitions with a stride-0 DMA. The V-bias
and output-projection bias enter as a precomputed (bv @ wout + bout) row.

Activation-table discipline: Sqrt (LN1) -> Exp (attention) -> Sqrt (LN2,
deferred until after all attention exps) -> Gelu (MLP), so the scalar engine
loads each activation table set at most twice.
"""

from contextlib import ExitStack

import numpy as np

P = 128
B, T, D, HD = 2, 2048, 1024, 64
H = 16
MLPD = 4096
EPS = 1e-5
NCORES = 8
GROUP = 4          # cores per batch group
HLOC = H // GROUP  # heads per core
TOK = T // GROUP   # tokens per core in the token-parallel part
NT = T // P        # 16 token tiles
ND = D // P        # 8 feature tiles
NSP = T // 512     # 4 query spans
NFF = MLPD // P    # 32

_cache = {}


def _classify_mask(attn_mask):
    """Per (span, kblock): 'skip' (fully masked), 'free', or 'partial'."""
    cls = {}
    for sp in range(NSP):
        qs = slice(sp * 512, sp * 512 + 512)
        for kb in range(NT):
            blk = attn_mask[qs, kb * P : kb * P + P]
            if blk.all():
                cls[(sp, kb)] = "skip"
            elif not blk.any():
                cls[(sp, kb)] = "free"
            else:
                cls[(sp, kb)] = "partial"
    return cls


def _mask_plan(attn_mask):
    """Dedupe partial-block mask tiles. Returns (mask_cls, tiles, index,
    off_map, hi_map): tiles is [n, 128, 512] f32 (additive, k-major),
    index[(sp, kb)] -> tile id, off_map -> count of leading fully-masked
    q-columns, hi_map -> 1 + last q-column with any masked element."""
    mask_cls = _classify_mask(attn_mask)
    tiles = []
    keys = {}
    index = {}
    off_map = {}
    hi_map = {}
    for (sp, kb), c in sorted(mask_cls.items()):
        if c != "partial":
            continue
        blk = np.asarray(attn_mask[sp * 512 : sp * 512 + 512,
                                   kb * P : kb * P + P]).T  # [128 k, 512 q]
        m = np.where(blk, np.float32(-1e30), np.float32(0.0))
        key = m.tobytes()
        if key not in keys:
            keys[key] = len(tiles)
            tiles.append(m)
        index[(sp, kb)] = keys[key]
        colmasked = blk.all(axis=0)  # [512] fully-masked q columns
        anymasked = blk.any(axis=0)
        off = 0
        while off < 512 and colmasked[off]:
            off += 1
        off_map[(sp, kb)] = off
        hi_map[(sp, kb)] = int(np.max(np.nonzero(anymasked)[0])) + 1
    if not tiles:
        tiles.append(np.zeros((P, 512), np.float32))
    return mask_cls, np.stack(tiles), index, off_map, hi_map


def _build(mask_cls, mask_index, n_mask, off_map, hi_map):
    import concourse.bass as bass
    import concourse.mybir as mybir
    import concourse.tile as tile
    from concourse import bacc
    from concourse.masks import make_identity

    f32 = mybir.dt.float32
    bf16 = mybir.dt.bfloat16
    AF = mybir.ActivationFunctionType
    OP = mybir.AluOpType

    nc = bacc.Bacc("TRN2", target_bir_lowering=False, debug=False,
                   num_devices=NCORES)

    x_b = nc.dram_tensor("x_b", [T, D], f32, kind="ExternalInput")
    x_tok = nc.dram_tensor("x_tok", [TOK, D], f32, kind="ExternalInput")
    wqkv_s = nc.dram_tensor("wqkv_s", [D, 3 * 256], bf16, kind="ExternalInput")
    bqk_s = nc.dram_tensor("bqk_s", [512], f32, kind="ExternalInput")
    wout_s = nc.dram_tensor("wout_s", [2 * P, D], bf16, kind="ExternalInput")
    bvout = nc.dram_tensor("bvout", [D], f32, kind="ExternalInput")
    w1 = nc.dram_tensor("w1", [D, MLPD], bf16, kind="ExternalInput")
    b1 = nc.dram_tensor("b1", [MLPD], f32, kind="ExternalInput")
    w2 = nc.dram_tensor("w2", [MLPD, D], bf16, kind="ExternalInput")
    b2 = nc.dram_tensor("b2", [D], f32, kind="ExternalInput")
    maskt = nc.dram_tensor("maskt", [n_mask * P, 512], f32,
                           kind="ExternalInput")
    out_tok = nc.dram_tensor("out_tok", [TOK, D], f32, kind="ExternalOutput")

    def bcast_ap(handle, n):
        a = handle.ap()
        return bass.AP(tensor=a.tensor, offset=a.offset, ap=[[0, P], [1, n]])

    with tile.TileContext(nc) as tc, ExitStack() as st:
        consts = st.enter_context(tc.tile_pool(name="consts", bufs=1))
        ident = consts.tile([P, P], f32)
        make_identity(nc, ident)

        def load_pmajor(vec, n, name):
            t = consts.tile([P, n], f32, name=name)
            nc.sync.dma_start(t[:], vec.ap().rearrange("(o p) -> p o", p=P))
            return t

        bqkt = load_pmajor(bqk_s, 4, "bqkt")
        b1m = load_pmajor(b1, NFF, "b1m")
        eps_t = consts.tile([P, 1], f32)
        nc.vector.memset(eps_t[:], EPS)
        ones_row = consts.tile([1, 64], f32)
        nc.vector.memset(ones_row[:], 1.0)
        b2_bc = consts.tile([P, D], f32)
        nc.gpsimd.dma_start(b2_bc[:], bcast_ap(b2, D))
        bvout_bc = consts.tile([P, D], f32)
        nc.gpsimd.dma_start(bvout_bc[:], bcast_ap(bvout, D))
        mask_sb = consts.tile([P, n_mask, 512], f32)
        # residual token rows for this core (interleaved spans)
        xr_sb = consts.tile([P, NSP, D], f32)

        dram = st.enter_context(tc.tile_pool(name="dram", bufs=1,
                                             space="DRAM"))
        partial_sp = [dram.tile([512, D], bf16, name=f"part{sp}")
                      for sp in range(NSP)]
        rs_sp = [dram.tile([P, D], bf16, name=f"rs{sp}")
                 for sp in range(NSP)]

        # persistent mid-state
        mid_pool = st.enter_context(tc.tile_pool(name="midp", bufs=1))
        x_mid = mid_pool.tile([P, NSP, D], f32)
        rs_f = mid_pool.tile([P, NSP, D], f32)

        # qv pool opened before hT so hT can be released first (LIFO)
        qv_st = ExitStack()
        qv_pool = qv_st.enter_context(tc.tile_pool(name="qv", bufs=1))
        qkT = qv_pool.tile([P, 4, T], bf16)   # [0,1]=q feats, [2,3]=k feats
        v_sb = qv_pool.tile([P, NT, HLOC, 65], bf16)
        ones_t = consts.tile([P, 1], f32, name="ones_t")
        nc.vector.memset(ones_t[:], 1.0)
        nc.vector.tensor_copy(
            v_sb[:, :, :, 64:65].rearrange("p a b c -> p (a b c)"),
            ones_t[:].to_broadcast((P, NT * HLOC)))
        wq_sb = qv_pool.tile([P, ND, 3 * 256], bf16)
        nc.gpsimd.dma_start(wq_sb[:],
                            wqkv_s.ap().rearrange("(o p) f -> p o f", p=P))
        wout_sb = qv_pool.tile([P, 2, D], bf16)
        nc.gpsimd.dma_start(wout_sb[:],
                            wout_s.ap().rearrange("(o p) f -> p o f", p=P))

        # ---------------- Stage A: LN1 + transpose -> hT ----------------
        # (ln1 gamma/beta are folded into wqkv/bvout on the host, so hT is
        # the plain normalized x-hat and the post-transpose op is a cast.)
        hT_st = ExitStack()
        hT_pool = hT_st.enter_context(tc.tile_pool(name="hT", bufs=1))
        hT = hT_pool.tile([P, ND, T], bf16)

        with tc.tile_pool(name="lnA", bufs=3) as lnA, \
             tc.tile_pool(name="lnAs", bufs=4) as lnAs, \
             tc.tile_pool(name="psA", bufs=4, space="PSUM") as psA:
            for tt in range(NT):
                x_sb = lnA.tile([P, D], f32, name="x_sb")
                nc.sync.dma_start(x_sb[:], x_b[tt * P : tt * P + P, :])
                stats = lnAs.tile([P, 2, 6], f32, name="stats")
                xg = x_sb[:].rearrange("p (g d) -> p g d", g=2)
                nc.vector.bn_stats(stats[:, 0, :], xg[:, 0, :])
                nc.vector.bn_stats(stats[:, 1, :], xg[:, 1, :])
                mv = lnAs.tile([P, 2], f32, name="mv")
                nc.vector.bn_aggr(mv[:], stats[:])
                rstd = lnAs.tile([P, 1], f32, name="rstd")
                nc.scalar.activation(rstd[:], mv[:, 1:2], AF.Sqrt,
                                     bias=eps_t[:])
                nc.vector.reciprocal(rstd[:], rstd[:])
                hno = lnA.tile([P, D], f32, name="hno")
                nc.vector.tensor_scalar(hno[:], x_sb[:], mv[:, 0:1], rstd[:],
                                        op0=OP.subtract, op1=OP.mult)
                for db in range(2):
                    ptr = psA.tile([P, 512], f32, name="ptr")
                    for dq in range(4):
                        dd = db * 4 + dq
                        nc.tensor.transpose(ptr[:, dq * P : dq * P + P],
                                            hno[:, dd * P : dd * P + P],
                                            ident[:])
                    nc.vector.tensor_copy(
                        hT[:, db * 4 : db * 4 + 4, tt * P : tt * P + P],
                        ptr[:].rearrange("p (c x) -> p c x", c=4))

        # deferred prefetches: issued after stage A's x loads so they don't
        # compete with the LN1 critical path for DMA bandwidth
        nc.sync.dma_start(mask_sb[:],
                          maskt.ap().rearrange("(o p) q -> p o q", p=P))
        nc.gpsimd.dma_start(xr_sb[:],
                            x_tok.ap().rearrange("(o p) f -> p o f", p=P))

        # ---------------- Stage B: qkT, V_aug ----------------
        with tc.tile_pool(name="psB", bufs=4, space="PSUM") as psB:
            for ft in range(4):
                for nb in range(4):
                    pq = psB.tile([P, 512], f32, name="pq")
                    for kk in range(ND):
                        nc.tensor.matmul(
                            pq[:], wq_sb[:, kk, ft * P : ft * P + P],
                            hT[:, kk, nb * 512 : nb * 512 + 512],
                            start=(kk == 0), stop=(kk == ND - 1))
                    nc.vector.tensor_scalar(
                        qkT[:, ft, nb * 512 : nb * 512 + 512], pq[:],
                        bqkt[:, ft : ft + 1], None, op0=OP.add)
            for tt in range(NT):
                pv = psB.tile([P, 256], f32, name="pv")
                for kk in range(ND):
                    nc.tensor.matmul(
                        pv[:], hT[:, kk, tt * P : tt * P + P],
                        wq_sb[:, kk, 512:768],
                        start=(kk == 0), stop=(kk == ND - 1))
                nc.vector.tensor_copy(
                    v_sb[:, tt, :, 0:64],
                    pv[:].rearrange("p (h d) -> p h d", h=HLOC))

        hT_st.close()

        # ---------------- Stage C: attention + per-span RS ----------------
        ctx_st = ExitStack()
        attn_pools = ctx_st.enter_context(tc.tile_pool(name="attn", bufs=3))
        attn2 = ctx_st.enter_context(tc.tile_pool(name="attn2", bufs=2))
        ctxp = ctx_st.enter_context(tc.tile_pool(name="ctxp", bufs=2))
        lnBs = ctx_st.enter_context(tc.tile_pool(name="lnBs", bufs=2))
        psS = ctx_st.enter_context(tc.tile_pool(name="psS", bufs=2,
                                                space="PSUM"))
        psC = ctx_st.enter_context(tc.tile_pool(name="psC", bufs=1,
                                                space="PSUM"))
        psW = ctx_st.enter_context(tc.tile_pool(name="psW", bufs=2,
                                                space="PSUM"))
        psN = ctx_st.enter_context(tc.tile_pool(name="psN", bufs=1,
                                                space="PSUM"))
        mvB_all = mid_pool.tile([P, NSP, 2], f32)

        def emit_d_stats(tl):
            # residual + LN2 stats for token tile tl -- vector only (no
            # PSUM, no scalar): anything placed here that waits on rs_f
            # stalls the queue it sits on, so the rsqrt happens post-C.
            nc.vector.tensor_add(x_mid[:, tl, :], rs_f[:, tl, :],
                                 bvout_bc[:])
            nc.vector.tensor_add(x_mid[:, tl, :], x_mid[:, tl, :],
                                 xr_sb[:, tl, :])
            stats = lnBs.tile([P, 2, 6], f32, name="statsB")
            xmg = x_mid[:, tl, :].rearrange("p (g d) -> p g d", g=2)
            nc.vector.bn_stats(stats[:, 0, :], xmg[:, 0, :])
            nc.vector.bn_stats(stats[:, 1, :], xmg[:, 1, :])
            nc.vector.bn_aggr(mvB_all[:, tl, :], stats[:])

        for sp in range(NSP):
            # Block descriptors (kb, off, mlo, mhi, mi): score/exp/PV over
            # q-columns [off, 512); additive mask over [mlo, mhi) from tile
            # mi. Partial (diagonal) blocks first; a full-width one leads
            # (PSUM start) and a full-width one ends (stop).
            partials = [kb for kb in range(NT)
                        if mask_cls[(sp, kb)] == "partial"]
            fulls = [kb for kb in range(NT) if mask_cls[(sp, kb)] == "free"]
            blocks = []
            for i, kb in enumerate(partials):
                off0, mhi = off_map[(sp, kb)], hi_map[(sp, kb)]
                mi = mask_index[(sp, kb)]
                if i == 0 or not fulls:
                    blocks.append((kb, 0, 0, mhi, mi))
                else:
                    blocks.append((kb, off0, off0, mhi, mi))
            blocks += [(kb, 0, 0, 0, None) for kb in fulls]
            nblk = len(blocks)
            assert blocks[0][1] == 0 and blocks[-1][1] == 0
            # pack into <=1024-column PSUM groups (one Exp per group); a
            # score matmul may not straddle a 512-col PSUM bank boundary
            groups = []
            cur = []
            curw = 0
            for blk in blocks:
                w = 512 - blk[1]
                fits = (curw + w <= 1024 and
                        ((curw % 512) + w <= 512 or
                         (curw % 512 == 0 and w == 512)))
                if not fits:
                    groups.append(cur)
                    cur, curw = [], 0
                cur.append((blk, curw))
                curw += w
            if cur:
                groups.append(cur)

            ctxT = ctxp.tile([P, 2, 512], bf16, name="ctxT")
            for h in range(HLOC):
                po = 64 * (h % 2)
                hq = h // 2
                pctx = psC.tile([P, 512], f32, name="pctx")

                def emit_scores(grp):
                    pair = psS.tile([P, 1024], f32, name="pair")
                    gw = 0
                    for (kb, off, mlo, mhi, mi), base in grp:
                        w = 512 - off
                        nc.tensor.matmul(
                            pair[:, base : base + w],
                            qkT[po : po + 64, 2 + hq, kb * P : kb * P + P],
                            qkT[po : po + 64, hq,
                                sp * 512 + off : sp * 512 + 512],
                            start=True, stop=True)
                        if mi is not None and mhi > mlo:
                            nc.vector.tensor_add(
                                pair[:, base + mlo - off : base + mhi - off],
                                pair[:, base + mlo - off : base + mhi - off],
                                mask_sb[:, mi, mlo:mhi])
                        gw = base + w
                    pT = attn_pools.tile([P, 1024], bf16, name="pT")
                    nc.scalar.activation(pT[:, 0:gw], pair[:, 0:gw], AF.Exp,
                                         scale=1.0 / float(np.sqrt(HD)))
                    return pT

                def emit_pv(grp, pT, i0):
                    i = i0
                    for (kb, off, mlo, mhi, mi), base in grp:
                        w = 512 - off
                        nc.tensor.matmul(
                            pctx[:65, off:512], v_sb[:, kb, h, :],
                            pT[:, base : base + w],
                            start=(i == 0), stop=(i == nblk - 1))
                        i += 1
                    return i

                # software pipeline: scores(g+1) emitted before PV(g)
                pTs = [emit_scores(groups[0])]
                done = 0
                for gi in range(1, len(groups)):
                    pTs.append(emit_scores(groups[gi]))
                    done = emit_pv(groups[gi - 1], pTs[gi - 1], done)
                done = emit_pv(groups[-1], pTs[-1], done)
                # normalize: denominator row -> SBUF, broadcast across 64
                # partitions with a ones-column matmul (PE -- keeps the
                # gpsimd queue free for the collectives), then reciprocal
                # and multiply on vector.
                dsb = attn2.tile([1, 512], f32, name="dsb")
                nc.vector.tensor_copy(dsb[:], pctx[64:65, :])
                dbb = psN.tile([64, 512], f32, name="dbb")
                nc.tensor.matmul(dbb[:], ones_row[:], dsb[:],
                                 start=True, stop=True)
                rbb = attn2.tile([64, 512], f32, name="rbb")
                nc.vector.reciprocal_approx_fast(rbb[:], dbb[:])
                nc.vector.tensor_mul(ctxT[po : po + 64, hq, :],
                                     pctx[0:64, :], rbb[:])
            # wout partial products for this span -> bf16 partials -> RS
            for tq in range(4):
                for fo in range(2):
                    pwo = psW.tile([P, 512], f32, name="pwo")
                    for kk in range(2):
                        nc.tensor.matmul(
                            pwo[:], ctxT[:, kk, tq * P : tq * P + P],
                            wout_sb[:, kk, fo * 512 : fo * 512 + 512],
                            start=(kk == 0), stop=(kk == 1))
                    po_sb = attn2.tile([P, 512], bf16, name="po_sb")
                    nc.scalar.copy(po_sb[:], pwo[:])
                    nc.sync.dma_start(
                        partial_sp[sp][tq * P : tq * P + P,
                                       fo * 512 : fo * 512 + 512], po_sb[:])
            nc.gpsimd.collective_compute(
                "ReduceScatter", mybir.AluOpType.add,
                ins=[partial_sp[sp].opt()], outs=[rs_sp[sp].opt()],
                replica_groups=[[0, 1, 2, 3], [4, 5, 6, 7]])
            nc.gpsimd.dma_start(rs_f[:, sp, :], rs_sp[sp][:])
            # LN2 stats lag two spans so they never wait on a live RS
            if sp >= 2:
                emit_d_stats(sp - 2)

        emit_d_stats(NSP - 2)
        emit_d_stats(NSP - 1)
        ctx_st.close()
        qv_st.close()

        # ------- Stage D: residual + LN2 -> h2T (sqrt deferred post-C) -----
        d_st = ExitStack()
        d_pool = d_st.enter_context(tc.tile_pool(name="dpool", bufs=1))
        h2T = d_pool.tile([P, ND, TOK], bf16)

        with tc.tile_pool(name="lnB", bufs=3) as lnB, \
             tc.tile_pool(name="psD2", bufs=4, space="PSUM") as psD2:
            for tl in range(NSP):
                nc.scalar.activation(mvB_all[:, tl, 1:2],
                                     mvB_all[:, tl, 1:2],
                                     AF.Sqrt, bias=eps_t[:])
                nc.vector.reciprocal(mvB_all[:, tl, 1:2],
                                     mvB_all[:, tl, 1:2])
            for tl in range(NSP):
                h2 = lnB.tile([P, D], f32, name="h2")
                nc.vector.tensor_scalar(h2[:], x_mid[:, tl, :],
                                        mvB_all[:, tl, 0:1],
                                        mvB_all[:, tl, 1:2],
                                        op0=OP.subtract, op1=OP.mult)
                for db in range(2):
                    ptr = psD2.tile([P, 512], f32, name="ptrD")
                    for dq in range(4):
                        dd = db * 4 + dq
                        nc.tensor.transpose(ptr[:, dq * P : dq * P + P],
                                            h2[:, dd * P : dd * P + P],
                                            ident[:])
                    nc.vector.tensor_copy(
                        h2T[:, db * 4 : db * 4 + 4, tl * P : tl * P + P],
                        ptr[:].rearrange("p (c x) -> p c x", c=4))

        # ---------------- Stage E: MLP ----------------
        mlp_st = ExitStack()
        mlp_pool = mlp_st.enter_context(tc.tile_pool(name="mlp", bufs=1))
        m1T = mlp_pool.tile([P, NFF, TOK], bf16)
        with tc.tile_pool(name="w1p", bufs=4) as w1p, \
             tc.tile_pool(name="psM1", bufs=2, space="PSUM") as psM1:
            for ff in range(NFF):
                w1c = w1p.tile([P, ND, P], bf16, name="w1c")
                nc.gpsimd.dma_start(
                    w1c[:], w1[:, ff * P : ff * P + P].rearrange(
                        "(o p) f -> p o f", p=P))
                pm1 = psM1.tile([P, 512], f32, name="pm1")
                for kk in range(ND):
                    nc.tensor.matmul(pm1[:], w1c[:, kk, :], h2T[:, kk, :],
                                     start=(kk == 0), stop=(kk == ND - 1))
                nc.scalar.activation(m1T[:, ff, :], pm1[:], AF.Gelu,
                                     bias=b1m[:, ff : ff + 1])

        with tc.tile_pool(name="w2p", bufs=3) as w2p, \
             tc.tile_pool(name="outp", bufs=3) as outp, \
             tc.tile_pool(name="psM2", bufs=1, space="PSUM") as psM2:
            pw2 = [[psM2.tile([P, 512], f32, name=f"pw2_{tl}_{fo}")
                    for fo in range(2)] for tl in range(NSP)]
            for ff in range(NFF):
                w2c = w2p.tile([P, D], bf16, name="w2c")
                nc.gpsimd.dma_start(w2c[:], w2[ff * P : ff * P + P, :])
                for tl in range(NSP):
                    for fo in range(2):
                        nc.tensor.matmul(
                            pw2[tl][fo][:], m1T[:, ff, tl * P : tl * P + P],
                            w2c[:, fo * 512 : fo * 512 + 512],
                            start=(ff == 0), stop=(ff == NFF - 1))
            for tl in range(NSP):
                for fo in range(2):
                    sl = slice(fo * 512, fo * 512 + 512)
                    o_sb = outp.tile([P, 512], f32, name="o_sb")
                    nc.vector.tensor_add(o_sb[:], pw2[tl][fo][:],
                                         x_mid[:, tl, sl])
                    nc.vector.tensor_add(o_sb[:], o_sb[:], b2_bc[:, sl])
                    nc.sync.dma_start(
                        out_tok[tl * P : tl * P + P, sl], o_sb[:])

        mlp_st.close()
        d_st.close()

    nc.compile()
    return nc


def _prepare_inputs(inputs):
    import ml_dtypes
    bf16 = ml_dtypes.bfloat16

    x = np.ascontiguousarray(np.asarray(inputs["x"], dtype=np.float32))
    attn_mask = np.asarray(inputs["attn_mask"])
    _, mask_tiles, _, _, _ = _mask_plan(attn_mask)

    # Fold LN1 gamma/beta into wqkv/bqkv and LN2 gamma/beta into w1/b1:
    #   ((xh*g + b) @ W = xh @ (g[:,None]*W) + b @ W    (exact, done in f64)
    g1 = np.asarray(inputs["ln1_g"], np.float64)
    be1 = np.asarray(inputs["ln1_b"], np.float64)
    g2 = np.asarray(inputs["ln2_g"], np.float64)
    be2 = np.asarray(inputs["ln2_b"], np.float64)
    wqkv0 = np.asarray(inputs["wqkv"], np.float64)
    bqkv0 = np.asarray(inputs["bqkv"], np.float64)
    wqkv = g1[:, None] * wqkv0
    bqkv = bqkv0 + be1 @ wqkv0
    w1_0 = np.asarray(inputs["w1"], np.float64)
    w1_eff = g2[:, None] * w1_0
    b1_eff = np.asarray(inputs["b1"], np.float64) + be2 @ w1_0

    wout_f = np.asarray(inputs["wout"], np.float64)
    bvout_full = (bqkv[2 * D : 3 * D] @ wout_f +
                  np.asarray(inputs["bout"], np.float64)).astype(np.float32)
    shared = {
        "bvout": bvout_full,
        "w1": np.ascontiguousarray(w1_eff.astype(np.float32).astype(bf16)),
        "b1": np.ascontiguousarray(b1_eff.astype(np.float32)),
        "w2": np.ascontiguousarray(np.asarray(inputs["w2"], np.float32)
                                   .astype(bf16)),
        "b2": np.ascontiguousarray(np.asarray(inputs["b2"], np.float32)),
        "maskt": np.ascontiguousarray(
            mask_tiles.reshape(-1, 512).astype(np.float32)),
    }
    wqkv = wqkv.astype(np.float32)
    bqkv = bqkv.astype(np.float32)
    in_maps = []
    for c in range(NCORES):
        b, r = divmod(c, GROUP)
        hs = slice(r * HLOC * HD, (r + 1) * HLOC * HD)  # 256 features
        wq = wqkv[:, 0:D][:, hs]
        wk = wqkv[:, D : 2 * D][:, hs]
        wv = wqkv[:, 2 * D : 3 * D][:, hs]
        # interleaved token ownership: spans x 128-row block r
        xrows = np.concatenate(
            [x[b, sp * 512 + r * P : sp * 512 + (r + 1) * P, :]
             for sp in range(NSP)], axis=0)
        in_maps.append(dict(
            shared,
            wout_s=np.ascontiguousarray(
                wout_f[hs, :].astype(np.float32).astype(bf16)),
            x_b=x[b],
            x_tok=np.ascontiguousarray(xrows),
            wqkv_s=np.ascontiguousarray(
                np.concatenate([wq, wk, wv], axis=1).astype(bf16)),
            bqk_s=np.ascontiguousarray(
                np.concatenate([bqkv[0:D][hs], bqkv[D : 2 * D][hs]])),
        ))
    return in_maps


def _get_nc(attn_mask):
    mask_cls, mask_tiles, mask_index, off_map, hi_map = _mask_plan(attn_mask)
    key = (tuple(sorted(mask_cls.items())), tuple(sorted(mask_index.items())),
           tuple(sorted(off_map.items())), tuple(sorted(hi_map.items())),
           mask_tiles.shape[0])
    if key not in _cache:
        _cache[key] = _build(mask_cls, mask_index, mask_tiles.shape[0],
                             off_map, hi_map)
    return _cache[key]


def run(inputs, trace=False):
    from concourse.bass_utils import run_bass_kernel_spmd

    attn_mask = np.asarray(inputs["attn_mask"])
    nc = _get_nc(attn_mask)
    in_maps = _prepare_inputs(inputs)
    kw = {}
    if trace:
        kw = dict(trace=True, trace_cores=list(range(NCORES)))
    res = run_bass_kernel_spmd(nc, in_maps, core_ids=list(range(NCORES)), **kw)
    out = np.empty((B, T, D), np.float32)
    for c in range(NCORES):
        b, r = divmod(c, GROUP)
        o = res.results[c]["out_tok"]
        for sp in range(NSP):
            out[b, sp * 512 + r * P : sp * 512 + (r + 1) * P, :] = \
                o[sp * P : (sp + 1) * P, :]
    return out, res


def kernel(**inputs):
    out, _ = run(inputs, trace=False)
    return out
